# revision 1
# baseline (speedup 1.0000x reference)
"""HSIC loss kernel for Trainium2, 8 NeuronCores.

Math: for each feature column c of X [2048, 16], K_c = rbf kernel matrix
(zero diag). Output = sum over feature pairs a<b of squared unbiased-HSIC
combination of T[a,b]=sum(K_a*K_b), rowsums A, total sums S.

Device strategy (SPMD over 8 cores, sharded over sample rows i):
  core r owns i in [r*256, (r+1)*256). Tiles are [128 j-partitions, 256 i-free].
  exponent(-gamma_c*(x_i-x_j)^2) built as:
     VE scalar_tensor_tensor: E = (xi - 2*xj)*xi = xi^2 - 2*xi*xj   (per-partition scalar 2*xj)
     ACT activation:          K = Exp(E*(-g_c) + (-g_c*xj^2)), accum_out -> per-j partial colsum
  K written bf16; TensorE accumulates the 16x16 Gram (trace) matrix over all
  (i,j) positions via [128, 16feat*8i] self-matmuls into one PSUM tile.
  Host combines per-core partials in float64.
"""

import sys
import numpy as np

if "/opt/trn_rl_repo" not in sys.path:
    sys.path.insert(0, "/opt/trn_rl_repo")

N = 2048
D = 16
P = 128
NCORES = 8
NI = N // NCORES          # 256 rows of i per core
NJB = N // P              # 16 j-blocks of 128 partitions
GJ = 8                    # i's packed per gram matmul -> D*GJ = 128 cols

_NC_CACHE = {}


def _patch_tile_drain():
    """Walrus in this container accepts only 1 sync-wait per instruction.
    Tile routinely attaches several. Hoist extra waits onto single-wait NoOp
    carriers emitted just before the instruction on the same engine, and
    split the tail drain's per-engine waits the same way."""
    import concourse.mybir as mybir
    import concourse.tile as tile_mod
    from concourse.vector_clock import ScopedClock, VectorClock

    if getattr(tile_mod.TileContext, "_drain_patched", False):
        return

    orig_add = tile_mod.TileContext._add_instruction
    counter = [0]

    def _add_instruction(self, inst):
        si = inst.sync_info
        if si is not None and si.on_wait is not None and len(si.on_wait) > 1:
            waits = list(si.on_wait)
            for w in waits[:-1]:
                counter[0] += 1
                carrier = mybir.InstNoOp(name=f"waitc-{counter[0]}")
                carrier.engine = inst.engine
                carrier.sync_info = mybir.SyncInfo(on_wait=[w], on_update=[])
                orig_add(self, carrier)
            inst.sync_info = mybir.SyncInfo(
                on_wait=[waits[-1]], on_update=list(si.on_update or [])
            )
        orig_add(self, inst)

    def _drain_and_barrier(self, tick_clock, wait_clock):
        vec = list(tick_clock.global_clock)
        for i, v in enumerate(vec):
            if v <= 0:
                continue
            sub = [v if j == i else 0 for j in range(len(vec))]
            carrier = self.nc.sync.nop(nofuse=True)
            wait_clock.add_sem_waits(
                carrier.ins, ScopedClock({None: VectorClock(sub)})
            )
        self.nc.sync.drain()
        self.nc.all_engine_barrier()
        popped = self.nc._tile_sem_poison_stack.pop()
        assert popped is self._sem_poison
        self.nc.clear_and_free_semaphores(list(self.sems.allocated().values()))
        self.nc.all_engine_barrier()

    tile_mod.TileContext._add_instruction = _add_instruction
    tile_mod.TileContext._drain_and_barrier = _drain_and_barrier
    tile_mod.TileContext._drain_patched = True


def _build_nc():
    import concourse.bass as bass
    import concourse.mybir as mybir
    from concourse.tile import TileContext

    _patch_tile_drain()

    f32 = mybir.dt.float32
    bf16 = mybir.dt.bfloat16

    nc = bass.Bass("TRN2")
    xi_rep_d = nc.dram_tensor("xi_rep", [P, D * NI], f32, kind="ExternalInput")
    sc2xj_d = nc.dram_tensor("sc2xj", [P, NJB * D], f32, kind="ExternalInput")
    bias_d = nc.dram_tensor("bias_t", [P, NJB * D], f32, kind="ExternalInput")
    gam_d = nc.dram_tensor("gam_t", [P, D], f32, kind="ExternalInput")
    apart_d = nc.dram_tensor("apart", [P, NJB * D], f32, kind="ExternalOutput")
    tpart_d = nc.dram_tensor("tpart", [P, P], f32, kind="ExternalOutput")

    n_mm = NJB * (NI // GJ)

    with TileContext(nc) as tc:
        with (
            tc.tile_pool(name="const", bufs=1) as cpool,
            tc.tile_pool(name="e", bufs=4) as epool,
            tc.tile_pool(name="k", bufs=2) as kpool,
            tc.tile_pool(name="ps", bufs=1, space="PSUM") as pspool,
        ):
            xi_rep = cpool.tile([P, D * NI], f32)
            sc2xj = cpool.tile([P, NJB * D], f32)
            bias_sb = cpool.tile([P, NJB * D], f32)
            gam_sb = cpool.tile([P, D], f32)
            abuf = cpool.tile([P, NJB * D], f32)
            tsb = cpool.tile([P, P], f32)

            nc.sync.dma_start(xi_rep[:], xi_rep_d[:])
            nc.sync.dma_start(sc2xj[:], sc2xj_d[:])
            nc.sync.dma_start(bias_sb[:], bias_d[:])
            nc.sync.dma_start(gam_sb[:], gam_d[:])

            gram = pspool.tile([P, P], f32)

            mm = 0
            for jb in range(NJB):
                # K stored i-major: column i*D + c, so gram operands are
                # contiguous [P, D*GJ] slices (matmul rhs needs 1 free dim).
                ktile = kpool.tile([P, D * NI], f32)
                k3 = ktile[:].rearrange("p (i c) -> p i c", c=D)
                for c in range(D):
                    e = epool.tile([P, NI], f32)
                    xi_c = xi_rep[:, c * NI : (c + 1) * NI]
                    col = jb * D + c
                    nc.vector.scalar_tensor_tensor(
                        out=e[:],
                        in0=xi_c,
                        scalar=sc2xj[:, col : col + 1],
                        in1=xi_c,
                        op0=mybir.AluOpType.subtract,
                        op1=mybir.AluOpType.mult,
                    )
                    nc.scalar.activation(
                        out=k3[:, :, c],
                        in_=e[:],
                        func=mybir.ActivationFunctionType.Exp,
                        bias=bias_sb[:, col : col + 1],
                        scale=gam_sb[:, c : c + 1],
                        accum_out=abuf[:, col : col + 1],
                    )
                for g in range(NI // GJ):
                    op = ktile[:, g * D * GJ : (g + 1) * D * GJ]
                    nc.tensor.matmul(
                        gram[:],
                        lhsT=op,
                        rhs=op,
                        start=(mm == 0),
                        stop=(mm == n_mm - 1),
                    )
                    mm += 1

            nc.vector.tensor_copy(tsb[:], gram[:])
            nc.sync.dma_start(apart_d[:], abuf[:])
            nc.sync.dma_start(tpart_d[:], tsb[:])
    return nc


def _get_nc():
    if "nc" not in _NC_CACHE:
        _NC_CACHE["nc"] = _build_nc()
    return _NC_CACHE["nc"]


def _make_in_maps(X):
    Xd = X.astype(np.float64)
    meanD = 2.0 * (np.mean(Xd * Xd, axis=0) - np.mean(Xd, axis=0) ** 2)  # [D]
    gamma = (1.0 / (2.0 * meanD)).astype(np.float32)  # 1/(2*sigma^2)

    xsq32 = X * X  # fl(x*x), matches device rounding
    bias_full = -(gamma[None, :] * xsq32)  # [N, D] f32
    sc2xj_full = 2.0 * X  # [N, D] f32

    def jblocked(a):  # [N, D] -> [P, NJB*D] with col jb*D+c = a[jb*128+p, c]
        return np.ascontiguousarray(
            a.reshape(NJB, P, D).transpose(1, 0, 2).reshape(P, NJB * D)
        )

    sc2xj = jblocked(sc2xj_full.astype(np.float32))
    bias_t = jblocked(bias_full.astype(np.float32))
    gam_t = np.ascontiguousarray(np.broadcast_to(-gamma[None, :], (P, D)))

    in_maps = []
    for r in range(NCORES):
        xi = X[r * NI : (r + 1) * NI, :]  # [NI, D]
        xi_rep = np.ascontiguousarray(
            np.broadcast_to(xi.T.reshape(1, D * NI), (P, D * NI))
        ).astype(np.float32)
        in_maps.append(
            {
                "xi_rep": xi_rep,
                "sc2xj": sc2xj,
                "bias_t": bias_t,
                "gam_t": gam_t.astype(np.float32),
            }
        )
    return in_maps


def _combine(results):
    A = np.zeros((D, N), dtype=np.float64)
    Tp = np.zeros((D, D), dtype=np.float64)
    for r in range(NCORES):
        ap = results[r]["apart"].astype(np.float64)  # [P, NJB*D]
        A += ap.reshape(P, NJB, D).transpose(2, 1, 0).reshape(D, N)
        tp = results[r]["tpart"].astype(np.float64).reshape(GJ, D, GJ, D)
        Tp += np.einsum("iaib->ab", tp)
    A -= 1.0  # remove diagonal K_ii = 1
    T = Tp - N  # remove sum_i K_ii^2
    S = A.sum(axis=1)
    Dm = A @ A.T
    c0 = 1.0 / (N * (N - 3))
    hsic = c0 * (
        T + np.outer(S, S) / ((N - 1.0) * (N - 2.0)) - (2.0 / (N - 2.0)) * Dm
    )
    iu = np.triu_indices(D, 1)
    return np.float32(np.sum(hsic[iu] ** 2))


def run_spmd(in_maps, **kwargs):
    from concourse import bass_utils

    nc = _get_nc()
    return bass_utils.run_bass_kernel_spmd(
        nc, in_maps, core_ids=list(range(NCORES)), **kwargs
    )


def kernel(X):
    X = np.ascontiguousarray(np.asarray(X, dtype=np.float32))
    in_maps = _make_in_maps(X)
    res = run_spmd(in_maps)
    return _combine(res.results)



# revision 8
# speedup vs baseline: 2.2849x; 2.2849x over previous
"""HSIC loss kernel for Trainium2, 8 NeuronCores — v2 (symmetric, fp16).

Math: X [2048, 16]; per feature column c, K_c = RBF kernel (zero diag);
output = sum over pairs a<b of squared unbiased-HSIC combination of
T[a,b]=sum(K_a*K_b), rowsums A, totals S.

v2 strategy (half the exp work of v1 via symmetry):
  K_c is symmetric, so only 136 of the 256 128x128 blocks are computed.
  Circulant assignment, uniform shapes: core r owns block-row r
  (covering column-blocks r..r+8 mod 16, 9 blocks) and block-row r+8
  (covering r+8..r+15 mod 16, 8 blocks) = 17 blocks/core.
  Per (row, i-chunk<=1024, feature c):
    TensorE: E = (-2*xj)*xi + xi^2 via K=2 fp16 matmul into PSUM
      (fp16 products are exact in f32 -> E is full precision of the
      fp16-rounded inputs; no replicated-x DMA, VectorE stays idle).
    ScalarE: K = Exp(-g_c*E + bias_j) PSUM->SBUF fp16 contiguous,
      accum_out -> f32 partial row-sums.
    TensorE: per 8-i group, fp16 gram matmul accumulated into G_diag /
      G_off PSUM tiles; off-diag groups also get a [128,1] ones-matmul
      = column sums (the transposed halves of A, via symmetry).
  Host (f64): A = row-sums + col-sums, T = Gd + 2*Go, exact diagonal
  correction, HSIC combination. fp16 keeps T and A consistent enough
  that the unbiased-estimator cancellation survives (~2e-4 rel err).
"""

import sys
import numpy as np

if "/opt/trn_rl_repo" not in sys.path:
    sys.path.insert(0, "/opt/trn_rl_repo")

N = 2048
D = 16
P = 128
NB = N // P               # 16 block-rows
NCORES = 8
LA, LB = 9, 8             # blocks covered by row A / row B
FLA, FLB = LA * P, LB * P  # 1152, 1024
NQ = (LA - 1) * D + (LB - 1) * D  # 240 off-diag groups per core
NSLOT = 3                 # ACT accum slots: (A,0:1024), (B,0:1024), (A,1024:1152)

_NC_CACHE = {}


def _patch_tile_drain():
    """Walrus in this container accepts only 1 sync-wait per instruction.
    Tile routinely attaches several. Hoist extra waits onto single-wait NoOp
    carriers emitted just before the instruction on the same engine, and
    split the tail drain's per-engine waits the same way."""
    import concourse.mybir as mybir
    import concourse.tile as tile_mod
    from concourse.vector_clock import ScopedClock, VectorClock

    if getattr(tile_mod.TileContext, "_drain_patched", False):
        return

    orig_add = tile_mod.TileContext._add_instruction
    counter = [0]

    def _add_instruction(self, inst):
        si = inst.sync_info
        if si is not None and si.on_wait is not None and len(si.on_wait) > 1:
            waits = list(si.on_wait)
            for w in waits[:-1]:
                counter[0] += 1
                carrier = mybir.InstNoOp(name=f"waitc-{counter[0]}")
                carrier.engine = inst.engine
                carrier.sync_info = mybir.SyncInfo(on_wait=[w], on_update=[])
                orig_add(self, carrier)
            inst.sync_info = mybir.SyncInfo(
                on_wait=[waits[-1]], on_update=list(si.on_update or [])
            )
        orig_add(self, inst)

    def _drain_and_barrier(self, tick_clock, wait_clock):
        vec = list(tick_clock.global_clock)
        for i, v in enumerate(vec):
            if v <= 0:
                continue
            sub = [v if j == i else 0 for j in range(len(vec))]
            carrier = self.nc.sync.nop(nofuse=True)
            wait_clock.add_sem_waits(
                carrier.ins, ScopedClock({None: VectorClock(sub)})
            )
        self.nc.sync.drain()
        self.nc.all_engine_barrier()
        popped = self.nc._tile_sem_poison_stack.pop()
        assert popped is self._sem_poison
        self.nc.clear_and_free_semaphores(list(self.sems.allocated().values()))
        self.nc.all_engine_barrier()

    tile_mod.TileContext._add_instruction = _add_instruction
    tile_mod.TileContext._drain_and_barrier = _drain_and_barrier
    tile_mod.TileContext._drain_patched = True


def _build_nc():
    import concourse.bass as bass
    import concourse.mybir as mybir
    from concourse.tile import TileContext

    _patch_tile_drain()

    f32 = mybir.dt.float32
    f16 = mybir.dt.float16

    nc = bass.Bass("TRN2")
    rx_d = nc.dram_tensor("rx", [2, D * (FLA + FLB)], f16, kind="ExternalInput")
    wj_d = nc.dram_tensor("wj", [2, 2 * D * P], f16, kind="ExternalInput")
    bias_d = nc.dram_tensor("bias_t", [P, 2 * D], f32, kind="ExternalInput")
    gam_d = nc.dram_tensor("gam_t", [P, D], f32, kind="ExternalInput")
    ones_d = nc.dram_tensor("ones_t", [P, 1], f16, kind="ExternalInput")
    gout_d = nc.dram_tensor("gout", [P, 2 * P], f32, kind="ExternalOutput")
    cs_d = nc.dram_tensor("cs", [P, NQ], f32, kind="ExternalOutput")
    rsum_d = nc.dram_tensor("rsum", [P, NSLOT * D], f32, kind="ExternalOutput")

    FL = {0: FLA, 1: FLB}
    # (row, col_start, col_end, accum_slot) in emission order; last chunk is
    # the small one so its gram tail is short.
    chunks = [(0, 0, 1024, 0), (1, 0, 1024, 1), (0, 1024, FLA, 2)]

    with TileContext(nc) as tc:
        with (
            tc.tile_pool(name="const", bufs=1) as cpool,
            tc.tile_pool(name="e", bufs=2, space="PSUM") as epool,
            tc.tile_pool(name="acc", bufs=1, space="PSUM") as apool,
        ):
            rx_sb = cpool.tile([2, D * (FLA + FLB)], f16)
            wj_sb = cpool.tile([2, 2 * D * P], f16)
            bias_sb = cpool.tile([P, 2 * D], f32)
            gam_sb = cpool.tile([P, D], f32)
            ones_sb = cpool.tile([P, 1], f16)
            ka_sb = cpool.tile([P, D * FLA], f16)
            kb_sb = cpool.tile([P, D * FLB], f16)
            rsum_sb = cpool.tile([P, NSLOT * D], f32)
            gout_sb = cpool.tile([P, 2 * P], f32)
            cs_sb = cpool.tile([P, NQ], f32)

            nc.sync.dma_start(rx_sb[:], rx_d[:])
            nc.sync.dma_start(wj_sb[:], wj_d[:])
            nc.sync.dma_start(bias_sb[:], bias_d[:])
            nc.sync.dma_start(gam_sb[:], gam_d[:])
            nc.sync.dma_start(ones_sb[:], ones_d[:])

            # one accumulation region per 2KB PSUM bank: a start=True matmul
            # clears has_written for the whole bank, so interleaved
            # accumulation groups must not share banks (pad tiles to 512 f32)
            gdps = apool.tile([P, 512], f32)     # [:, :128] = G_diag
            gops = apool.tile([P, 512], f32)     # [:, :128] = G_off
            csps = apool.tile([P, 512], f32)     # [:, :NQ]  = col sums

            ksb = {0: ka_sb, 1: kb_sb}
            rxoff = {0: 0, 1: D * FLA}
            n_diag = 2 * D                        # 32 diag gram matmuls
            n_off = NQ                            # 240 off gram matmuls
            di = [0]
            oi = [0]

            for (row, s, e, slot) in chunks:
                fl = FL[row]
                w = e - s
                et = epool.tile([P, 1024], f32)
                for c in range(D):
                    # E = (-2*xj)*xi + xi^2, K=2 contraction, 512-col pieces
                    # (matmul PSUM-out must stay within one 2KB bank).
                    lw = wj_sb[0:2, (row * D + c) * P : (row * D + c + 1) * P]
                    for ps in range(s, e, 512):
                        pe_ = min(ps + 512, e)
                        nc.tensor.matmul(
                            et[:, ps - s : pe_ - s],
                            lhsT=lw,
                            rhs=rx_sb[0:2, rxoff[row] + c * fl + ps : rxoff[row] + c * fl + pe_],
                            start=True,
                            stop=True,
                        )
                    # K layout: col = g*128 + c*8 + ii (group-interleaved) so
                    # gram operands are contiguous 1-D 128-col slices; the ACT
                    # write scatters 8-elem (16B) runs.
                    k3 = ksb[row][:].rearrange("p (g x) -> p g x", x=D * 8)
                    nc.scalar.activation(
                        out=k3[:, s // 8 : e // 8, c * 8 : (c + 1) * 8],
                        in_=et[:, :w],
                        func=mybir.ActivationFunctionType.Exp,
                        bias=bias_sb[:, row * D + c : row * D + c + 1],
                        scale=gam_sb[:, c : c + 1],
                        accum_out=rsum_sb[:, slot * D + c : slot * D + c + 1],
                    )
                # gram + colsum matmuls for this chunk's 8-i groups
                for g in range(s // 8, e // 8):
                    op = ksb[row][:, g * 128 : (g + 1) * 128]
                    if g < 16:
                        nc.tensor.matmul(
                            gdps[:, 0:P],
                            lhsT=op,
                            rhs=op,
                            start=(di[0] == 0),
                            stop=(di[0] == n_diag - 1),
                            skip_group_check=True,
                        )
                        di[0] += 1
                    else:
                        nc.tensor.matmul(
                            gops[:, 0:P],
                            lhsT=op,
                            rhs=op,
                            start=(oi[0] == 0),
                            stop=(oi[0] == n_off - 1),
                            skip_group_check=True,
                        )
                        q = (g - 16) if row == 0 else (LA - 1) * D + (g - 16)
                        nc.tensor.matmul(
                            csps[:, q : q + 1],
                            lhsT=op,
                            rhs=ones_sb[:, 0:1],
                            start=True,
                            stop=True,
                            skip_group_check=True,
                        )
                        oi[0] += 1

            nc.vector.tensor_copy(gout_sb[:, 0:P], gdps[:, 0:P])
            nc.vector.tensor_copy(gout_sb[:, P : 2 * P], gops[:, 0:P])
            nc.vector.tensor_copy(cs_sb[:], csps[:, 0:NQ])
            nc.sync.dma_start(gout_d[:], gout_sb[:])
            nc.sync.dma_start(cs_d[:], cs_sb[:])
            nc.sync.dma_start(rsum_d[:], rsum_sb[:])
    return nc


def _get_nc():
    if "nc" not in _NC_CACHE:
        _NC_CACHE["nc"] = _build_nc()
    return _NC_CACHE["nc"]


def _prep(X):
    """Host-side constants shared by in-map prep and combine."""
    Xd = X.astype(np.float64)
    meanD = 2.0 * (np.mean(Xd * Xd, axis=0) - np.mean(Xd, axis=0) ** 2)
    g32 = (1.0 / (2.0 * meanD)).astype(np.float32)       # [D]
    x16 = X.astype(np.float16).astype(np.float32)        # \tilde x
    xsq16 = (x16 * x16).astype(np.float16).astype(np.float32)
    return g32, x16, xsq16


def _make_in_maps(X):
    _COMBINE_X[0] = np.ascontiguousarray(np.asarray(X, dtype=np.float32))
    g32, x16, xsq16 = _prep(X)
    bias_full = -(g32[None, :] * xsq16).astype(np.float32)   # [N, D]

    in_maps = []
    for r in range(NCORES):
        rows = [(r, FLA), (r + 8, FLB)]
        # rx: per row, c-major slabs of x / x^2 along the wrapped i-range
        rx = np.zeros((2, D * (FLA + FLB)), dtype=np.float16)
        off = 0
        for (J, fl) in rows:
            idx = (J * P + np.arange(fl)) % N
            for c in range(D):
                rx[0, off + c * fl : off + (c + 1) * fl] = x16[idx, c]
                rx[1, off + c * fl : off + (c + 1) * fl] = xsq16[idx, c]
            off += D * fl
        # wj: [2, (row*D+c)*P + j]: row0 = -2*xj, row1 = 1
        wj = np.zeros((2, 2 * D * P), dtype=np.float16)
        bias = np.zeros((P, 2 * D), dtype=np.float32)
        for row, (J, fl) in enumerate(rows):
            jidx = J * P + np.arange(P)
            for c in range(D):
                wj[0, (row * D + c) * P : (row * D + c + 1) * P] = -2.0 * x16[jidx, c]
                wj[1, (row * D + c) * P : (row * D + c + 1) * P] = 1.0
                bias[:, row * D + c] = bias_full[jidx, c]
        gam = np.ascontiguousarray(
            np.broadcast_to(-g32[None, :], (P, D))
        ).astype(np.float32)
        ones = np.ones((P, 1), dtype=np.float16)
        in_maps.append(
            {"rx": rx, "wj": wj, "bias_t": bias, "gam_t": gam, "ones_t": ones}
        )
    return in_maps


def _combine(results, X=None):
    if X is None:
        X = _COMBINE_X[0]
    g32, x16, xsq16 = _prep(X)
    g64 = g32.astype(np.float64)

    # exact diagonal model: E_ii = -2*x^2 + q(x^2) (f32-exact products),
    # arg = fma(E, -g, -g*q(x^2)), K_ii = exp(arg)
    E_ii = (-2.0 * (x16.astype(np.float64) ** 2) + xsq16).astype(np.float32)
    bias_full = -(g32[None, :] * xsq16).astype(np.float32)
    arg = (
        -g64[None, :] * E_ii.astype(np.float64) + bias_full.astype(np.float64)
    ).astype(np.float32)
    Kii = np.exp(arg.astype(np.float64))                  # [N, D]

    A = np.zeros((D, N), dtype=np.float64)
    Tp = np.zeros((D, D), dtype=np.float64)
    for r in range(NCORES):
        res = results[r]
        rsum = res["rsum"].astype(np.float64)             # [P, 3*D]
        cs = res["cs"].astype(np.float64)                 # [P, 240]
        gout = res["gout"].astype(np.float64)             # [P, 256]
        rows = [(r, FLA), (r + 8, FLB)]
        # row-sums: slots 0,2 -> row A; slot 1 -> row B
        A[:, r * P : (r + 1) * P] += (rsum[:, 0:D] + rsum[:, 2 * D : 3 * D]).T
        A[:, (r + 8) * P : (r + 9) * P] += rsum[:, D : 2 * D].T
        # col-sums: partition p = c*8 + ii; col q = off-group index
        csv = cs.reshape(D, 8, NQ)                        # [c, ii, q]
        for row, (J, fl) in enumerate(rows):
            nq = (LA - 1) * D if row == 0 else (LB - 1) * D
            qb = 0 if row == 0 else (LA - 1) * D
            q0 = np.arange(nq)
            ii = np.arange(8)
            cols = P + q0[:, None] * 8 + ii[None, :]      # [nq, 8] within-row col
            idx = (J * P + cols) % N
            A[:, idx.ravel()] += csv[:, :, qb : qb + nq].transpose(
                0, 2, 1
            ).reshape(D, -1)
        gd = gout[:, :P].reshape(D, 8, D, 8)
        go = gout[:, P:].reshape(D, 8, D, 8)
        Tp += np.einsum("aibi->ab", gd) + 2.0 * np.einsum("aibi->ab", go)

    A -= Kii.T
    T = Tp - Kii.T @ Kii
    S = A.sum(axis=1)
    Dm = A @ A.T
    c0 = 1.0 / (N * (N - 3))
    hsic = c0 * (
        T + np.outer(S, S) / ((N - 1.0) * (N - 2.0)) - (2.0 / (N - 2.0)) * Dm
    )
    iu = np.triu_indices(D, 1)
    return np.float32(np.sum(hsic[iu] ** 2))


_COMBINE_X = [None]


def run_spmd(in_maps, **kwargs):
    from concourse import bass_utils

    nc = _get_nc()
    return bass_utils.run_bass_kernel_spmd(
        nc, in_maps, core_ids=list(range(NCORES)), **kwargs
    )


def kernel(X):
    X = np.ascontiguousarray(np.asarray(X, dtype=np.float32))
    _COMBINE_X[0] = X
    in_maps = _make_in_maps(X)
    res = run_spmd(in_maps)
    return _combine(res.results, X)


# revision 12
# speedup vs baseline: 2.4115x; 1.0554x over previous
"""HSIC loss kernel for Trainium2, 8 NeuronCores — v2 (symmetric, fp16).

Math: X [2048, 16]; per feature column c, K_c = RBF kernel (zero diag);
output = sum over pairs a<b of squared unbiased-HSIC combination of
T[a,b]=sum(K_a*K_b), rowsums A, totals S.

v2 strategy (half the exp work of v1 via symmetry):
  K_c is symmetric, so only 136 of the 256 128x128 blocks are computed.
  Circulant assignment, uniform shapes: core r owns block-row r
  (covering column-blocks r..r+8 mod 16, 9 blocks) and block-row r+8
  (covering r+8..r+15 mod 16, 8 blocks) = 17 blocks/core.
  Per (row, i-chunk<=1024, feature c):
    TensorE: E = (-2*xj)*xi + xi^2 via K=2 fp16 matmul into PSUM
      (fp16 products are exact in f32 -> E is full precision of the
      fp16-rounded inputs; no replicated-x DMA, VectorE stays idle).
    ScalarE: K = Exp(-g_c*E + bias_j) PSUM->SBUF fp16 contiguous,
      accum_out -> f32 partial row-sums.
    TensorE: per 8-i group, fp16 gram matmul accumulated into G_diag /
      G_off PSUM tiles; off-diag groups also get a [128,1] ones-matmul
      = column sums (the transposed halves of A, via symmetry).
  Host (f64): A = row-sums + col-sums, T = Gd + 2*Go, exact diagonal
  correction, HSIC combination. fp16 keeps T and A consistent enough
  that the unbiased-estimator cancellation survives (~2e-4 rel err).
"""

import sys
import numpy as np

if "/opt/trn_rl_repo" not in sys.path:
    sys.path.insert(0, "/opt/trn_rl_repo")

N = 2048
D = 16
P = 128
NB = N // P               # 16 block-rows
NCORES = 8
LA, LB = 9, 8             # blocks covered by row A / row B
FLA, FLB = LA * P, LB * P  # 1152, 1024
NQ = (LA - 1) * D + (LB - 1) * D  # 240 off-diag groups per core
NSLOT = 3                 # ACT accum slots: (A,0:1024), (B,0:1024), (A,1024:1152)

_NC_CACHE = {}


def _patch_tile_drain():
    """Walrus in this container accepts only 1 sync-wait per instruction.
    Tile routinely attaches several. Hoist extra waits onto single-wait NoOp
    carriers emitted just before the instruction on the same engine, and
    split the tail drain's per-engine waits the same way."""
    import concourse.mybir as mybir
    import concourse.tile as tile_mod
    from concourse.vector_clock import ScopedClock, VectorClock

    if getattr(tile_mod.TileContext, "_drain_patched", False):
        return

    orig_add = tile_mod.TileContext._add_instruction
    counter = [0]

    def _add_instruction(self, inst):
        si = inst.sync_info
        if si is not None and si.on_wait is not None and len(si.on_wait) > 1:
            waits = list(si.on_wait)
            for w in waits[:-1]:
                counter[0] += 1
                carrier = mybir.InstNoOp(name=f"waitc-{counter[0]}")
                carrier.engine = inst.engine
                carrier.sync_info = mybir.SyncInfo(on_wait=[w], on_update=[])
                orig_add(self, carrier)
            inst.sync_info = mybir.SyncInfo(
                on_wait=[waits[-1]], on_update=list(si.on_update or [])
            )
        orig_add(self, inst)

    def _drain_and_barrier(self, tick_clock, wait_clock):
        vec = list(tick_clock.global_clock)
        for i, v in enumerate(vec):
            if v <= 0:
                continue
            sub = [v if j == i else 0 for j in range(len(vec))]
            carrier = self.nc.sync.nop(nofuse=True)
            wait_clock.add_sem_waits(
                carrier.ins, ScopedClock({None: VectorClock(sub)})
            )
        self.nc.sync.drain()
        self.nc.all_engine_barrier()
        popped = self.nc._tile_sem_poison_stack.pop()
        assert popped is self._sem_poison
        self.nc.clear_and_free_semaphores(list(self.sems.allocated().values()))
        self.nc.all_engine_barrier()

    tile_mod.TileContext._add_instruction = _add_instruction
    tile_mod.TileContext._drain_and_barrier = _drain_and_barrier
    tile_mod.TileContext._drain_patched = True


def _build_nc():
    import concourse.bass as bass
    import concourse.mybir as mybir
    from concourse.tile import TileContext

    _patch_tile_drain()

    f32 = mybir.dt.float32
    f16 = mybir.dt.float16

    nc = bass.Bass("TRN2")
    rx_d = nc.dram_tensor("rx", [2, D * (FLA + FLB)], f16, kind="ExternalInput")
    wj_d = nc.dram_tensor("wj", [2, 2 * D * P], f16, kind="ExternalInput")
    bias_d = nc.dram_tensor("bias_t", [P, 2 * D], f32, kind="ExternalInput")
    gam_d = nc.dram_tensor("gam_t", [P, D], f32, kind="ExternalInput")
    ones_d = nc.dram_tensor("ones_t", [P, 1], f16, kind="ExternalInput")
    gout_d = nc.dram_tensor("gout", [P, 2 * P], f32, kind="ExternalOutput")
    cs_d = nc.dram_tensor("cs", [P, NQ], f32, kind="ExternalOutput")
    rsum_d = nc.dram_tensor("rsum", [P, NSLOT * D], f32, kind="ExternalOutput")

    FL = {0: FLA, 1: FLB}
    # (row, col_start, col_end, accum_slot) in emission order; last chunk is
    # the small one so its gram tail is short.
    chunks = [(0, 0, 1024, 0), (1, 0, 1024, 1), (0, 1024, FLA, 2)]

    with TileContext(nc) as tc:
        with (
            tc.tile_pool(name="const", bufs=1) as cpool,
            tc.tile_pool(name="e", bufs=2, space="PSUM") as epool,
            tc.tile_pool(name="acc", bufs=1, space="PSUM") as apool,
        ):
            rx_sb = cpool.tile([2, D * (FLA + FLB)], f16)
            wj_sb = cpool.tile([2, 2 * D * P], f16)
            bias_sb = cpool.tile([P, 2 * D], f32)
            gam_sb = cpool.tile([P, D], f32)
            ones_sb = cpool.tile([P, 1], f16)
            ka_sb = cpool.tile([P, D * FLA], f16)
            kb_sb = cpool.tile([P, D * FLB], f16)
            rsum_sb = cpool.tile([P, NSLOT * D], f32)
            gout_sb = cpool.tile([P, 2 * P], f32)
            cs_sb = cpool.tile([P, NQ], f32)

            scr_sb = cpool.tile([P, 1], f32)

            nc.sync.dma_start(ones_sb[:], ones_d[:])
            nc.sync.dma_start(rx_sb[:], rx_d[:])
            nc.sync.dma_start(wj_sb[:], wj_d[:])
            nc.sync.dma_start(bias_sb[:], bias_d[:])
            nc.sync.dma_start(gam_sb[:], gam_d[:])

            # one accumulation region per 2KB PSUM bank: a start=True matmul
            # clears has_written for the whole bank, so interleaved
            # accumulation groups must not share banks (pad tiles to 512 f32)
            gdps = apool.tile([P, 512], f32)     # [:, :128] = G_diag
            gops = apool.tile([P, 512], f32)     # [:, :128] = G_off
            csps = apool.tile([P, 512], f32)     # [:, :NQ]  = col sums
            wups = apool.tile([P, 512], f32)     # warmup/scratch bank

            # early exp-table load (overlaps input DMA) and PE warmup so the
            # HAM clock-gate is at 8/8 before the real matmuls arrive
            nc.scalar.activation(
                out=scr_sb[:],
                in_=ones_sb[:],
                func=mybir.ActivationFunctionType.Exp,
            )
            for wu in range(64):
                nc.tensor.matmul(
                    wups[0:1, wu % 8 : wu % 8 + 1],
                    lhsT=ones_sb[:, 0:1],
                    rhs=ones_sb[:, 0:1],
                    start=True,
                    stop=True,
                    skip_group_check=True,
                )

            ksb = {0: ka_sb, 1: kb_sb}
            rxoff = {0: 0, 1: D * FLA}
            n_diag = 2 * D                        # 32 diag gram matmuls
            n_off = NQ                            # 240 off gram matmuls
            di = [0]
            oi = [0]

            pend = []  # deferred gram/cs matmul thunks from finished chunks

            def mk_gram(row, g):
                def thunk():
                    op = ksb[row][:, g * 128 : (g + 1) * 128]
                    if g < 16:
                        nc.tensor.matmul(
                            gdps[:, 0:P], lhsT=op, rhs=op,
                            start=(di[0] == 0), stop=(di[0] == n_diag - 1),
                            skip_group_check=True,
                        )
                        di[0] += 1
                    else:
                        nc.tensor.matmul(
                            gops[:, 0:P], lhsT=op, rhs=op,
                            start=(oi[0] == 0), stop=(oi[0] == n_off - 1),
                            skip_group_check=True,
                        )
                        q = (g - 16) if row == 0 else (LA - 1) * D + (g - 16)
                        nc.tensor.matmul(
                            csps[:, q : q + 1], lhsT=op, rhs=ones_sb[:, 0:1],
                            start=True, stop=True, skip_group_check=True,
                        )
                        oi[0] += 1
                return thunk

            for (row, s, e, slot) in chunks:
                fl = FL[row]
                w = e - s
                et = epool.tile([P, 1024], f32)
                per_slot = -(-len(pend) // D)  # spread pending over 16 c-slots
                for c in range(D):
                    # E = (-2*xj)*xi + xi^2, K=2 contraction, 512-col pieces
                    # (matmul PSUM-out must stay within one 2KB bank).
                    lw = wj_sb[0:2, (row * D + c) * P : (row * D + c + 1) * P]
                    for ps in range(s, e, 512):
                        pe_ = min(ps + 512, e)
                        nc.tensor.matmul(
                            et[:, ps - s : pe_ - s],
                            lhsT=lw,
                            rhs=rx_sb[0:2, rxoff[row] + c * fl + ps : rxoff[row] + c * fl + pe_],
                            start=True,
                            stop=True,
                        )
                    # K layout: col = g*128 + c*8 + ii (group-interleaved) so
                    # gram operands are contiguous 1-D 128-col slices; the ACT
                    # write scatters 8-elem (16B) runs.
                    k3 = ksb[row][:].rearrange("p (g x) -> p g x", x=D * 8)
                    nc.scalar.activation(
                        out=k3[:, s // 8 : e // 8, c * 8 : (c + 1) * 8],
                        in_=et[:, :w],
                        func=mybir.ActivationFunctionType.Exp,
                        bias=bias_sb[:, row * D + c : row * D + c + 1],
                        scale=gam_sb[:, c : c + 1],
                        accum_out=rsum_sb[:, slot * D + c : slot * D + c + 1],
                    )
                    # drain deferred gram/cs work between E matmuls so the
                    # PE stream stays dense (hides SBUF access latency) and
                    # never convoys behind a whole chunk's gram phase
                    for _ in range(min(per_slot, len(pend))):
                        pend.pop(0)()
                # queue this chunk's gram/cs matmuls (they need all 16 c)
                for g in range(s // 8, e // 8):
                    pend.append(mk_gram(row, g))
            while pend:
                pend.pop(0)()

            nc.vector.tensor_copy(gout_sb[:, 0:P], gdps[:, 0:P])
            nc.vector.tensor_copy(gout_sb[:, P : 2 * P], gops[:, 0:P])
            nc.vector.tensor_copy(cs_sb[:], csps[:, 0:NQ])
            nc.sync.dma_start(gout_d[:], gout_sb[:])
            nc.sync.dma_start(cs_d[:], cs_sb[:])
            nc.sync.dma_start(rsum_d[:], rsum_sb[:])
    return nc


def _get_nc():
    if "nc" not in _NC_CACHE:
        _NC_CACHE["nc"] = _build_nc()
    return _NC_CACHE["nc"]


def _prep(X):
    """Host-side constants shared by in-map prep and combine."""
    Xd = X.astype(np.float64)
    meanD = 2.0 * (np.mean(Xd * Xd, axis=0) - np.mean(Xd, axis=0) ** 2)
    g32 = (1.0 / (2.0 * meanD)).astype(np.float32)       # [D]
    x16 = X.astype(np.float16).astype(np.float32)        # \tilde x
    xsq16 = (x16 * x16).astype(np.float16).astype(np.float32)
    return g32, x16, xsq16


def _make_in_maps(X):
    _COMBINE_X[0] = np.ascontiguousarray(np.asarray(X, dtype=np.float32))
    g32, x16, xsq16 = _prep(X)
    bias_full = -(g32[None, :] * xsq16).astype(np.float32)   # [N, D]

    in_maps = []
    for r in range(NCORES):
        rows = [(r, FLA), (r + 8, FLB)]
        # rx: per row, c-major slabs of x / x^2 along the wrapped i-range
        rx = np.zeros((2, D * (FLA + FLB)), dtype=np.float16)
        off = 0
        for (J, fl) in rows:
            idx = (J * P + np.arange(fl)) % N
            for c in range(D):
                rx[0, off + c * fl : off + (c + 1) * fl] = x16[idx, c]
                rx[1, off + c * fl : off + (c + 1) * fl] = xsq16[idx, c]
            off += D * fl
        # wj: [2, (row*D+c)*P + j]: row0 = -2*xj, row1 = 1
        wj = np.zeros((2, 2 * D * P), dtype=np.float16)
        bias = np.zeros((P, 2 * D), dtype=np.float32)
        for row, (J, fl) in enumerate(rows):
            jidx = J * P + np.arange(P)
            for c in range(D):
                wj[0, (row * D + c) * P : (row * D + c + 1) * P] = -2.0 * x16[jidx, c]
                wj[1, (row * D + c) * P : (row * D + c + 1) * P] = 1.0
                bias[:, row * D + c] = bias_full[jidx, c]
        gam = np.ascontiguousarray(
            np.broadcast_to(-g32[None, :], (P, D))
        ).astype(np.float32)
        ones = np.ones((P, 1), dtype=np.float16)
        in_maps.append(
            {"rx": rx, "wj": wj, "bias_t": bias, "gam_t": gam, "ones_t": ones}
        )
    return in_maps


def _combine(results, X=None):
    if X is None:
        X = _COMBINE_X[0]
    g32, x16, xsq16 = _prep(X)
    g64 = g32.astype(np.float64)

    # exact diagonal model: E_ii = -2*x^2 + q(x^2) (f32-exact products),
    # arg = fma(E, -g, -g*q(x^2)), K_ii = exp(arg)
    E_ii = (-2.0 * (x16.astype(np.float64) ** 2) + xsq16).astype(np.float32)
    bias_full = -(g32[None, :] * xsq16).astype(np.float32)
    arg = (
        -g64[None, :] * E_ii.astype(np.float64) + bias_full.astype(np.float64)
    ).astype(np.float32)
    Kii = np.exp(arg.astype(np.float64))                  # [N, D]

    A = np.zeros((D, N), dtype=np.float64)
    Tp = np.zeros((D, D), dtype=np.float64)
    for r in range(NCORES):
        res = results[r]
        rsum = res["rsum"].astype(np.float64)             # [P, 3*D]
        cs = res["cs"].astype(np.float64)                 # [P, 240]
        gout = res["gout"].astype(np.float64)             # [P, 256]
        rows = [(r, FLA), (r + 8, FLB)]
        # row-sums: slots 0,2 -> row A; slot 1 -> row B
        A[:, r * P : (r + 1) * P] += (rsum[:, 0:D] + rsum[:, 2 * D : 3 * D]).T
        A[:, (r + 8) * P : (r + 9) * P] += rsum[:, D : 2 * D].T
        # col-sums: partition p = c*8 + ii; col q = off-group index
        csv = cs.reshape(D, 8, NQ)                        # [c, ii, q]
        for row, (J, fl) in enumerate(rows):
            nq = (LA - 1) * D if row == 0 else (LB - 1) * D
            qb = 0 if row == 0 else (LA - 1) * D
            q0 = np.arange(nq)
            ii = np.arange(8)
            cols = P + q0[:, None] * 8 + ii[None, :]      # [nq, 8] within-row col
            idx = (J * P + cols) % N
            A[:, idx.ravel()] += csv[:, :, qb : qb + nq].transpose(
                0, 2, 1
            ).reshape(D, -1)
        gd = gout[:, :P].reshape(D, 8, D, 8)
        go = gout[:, P:].reshape(D, 8, D, 8)
        Tp += np.einsum("aibi->ab", gd) + 2.0 * np.einsum("aibi->ab", go)

    A -= Kii.T
    T = Tp - Kii.T @ Kii
    S = A.sum(axis=1)
    Dm = A @ A.T
    c0 = 1.0 / (N * (N - 3))
    hsic = c0 * (
        T + np.outer(S, S) / ((N - 1.0) * (N - 2.0)) - (2.0 / (N - 2.0)) * Dm
    )
    iu = np.triu_indices(D, 1)
    return np.float32(np.sum(hsic[iu] ** 2))


_COMBINE_X = [None]


def run_spmd(in_maps, **kwargs):
    from concourse import bass_utils

    nc = _get_nc()
    return bass_utils.run_bass_kernel_spmd(
        nc, in_maps, core_ids=list(range(NCORES)), **kwargs
    )


def kernel(X):
    X = np.ascontiguousarray(np.asarray(X, dtype=np.float32))
    _COMBINE_X[0] = X
    in_maps = _make_in_maps(X)
    res = run_spmd(in_maps)
    return _combine(res.results, X)
